# revision 1
# baseline (speedup 1.0000x reference)
"""CVMerge scatter kernel for Trainium2 (8 NeuronCores, data-parallel).

Reference semantics: fold = arange(N) % 4 (static), so the scatter
    out[4*j + i] = x_i[j]
is a pure deterministic interleave of four [K, 32] f32 arrays into
[N, 32].  Row-parallel split across 8 cores: core c handles j in
[c*J, (c+1)*J), J = K/8, producing output rows [c*4J, (c+1)*4J).

Production variant ("computeq", chosen by HW A/B benchmarking):
per core, tile over j (JT=8192 j-groups per tile).  For each tile:
  - 4 load DMAs (HWDGE, SP ring) read each x_i's contiguous [JT, 32]
    DRAM block into its own contiguous SBUF region,
  - 4 DVE tensor_copy ops (two folds x two q-halves each, 4D strided
    APs) interleave the regions into a second SBUF tile laid out
    exactly as the output block,
  - 2 store DMAs (HWDGE, ACT ring — a different ring than the loads
    so a waiting store cannot head-of-line-block later loads) write
    the q-halves to contiguous DRAM.
Both HBM sides are fully contiguous (≥1 MB transfers); the interleave
lives entirely in SBUF where the DVE handles the 256B-chunk strides.
Measured ~150 us/core steady state ≈ 430+ GB/s/core of combined HBM
read+write traffic, within a few percent of the empirical roofline of
an equivalent contiguous memcpy on this machine.
"""

import numpy as np

N = 2097152          # total output rows
NF = 4               # folds
K = N // NF          # rows per fold = 524288
D = 32               # feature dim
NCORES = 8
J = K // NCORES      # j-groups per core = 65536
JT = 8192            # j-groups per tile
T = J // JT          # tiles per core = 8
QT = JT // 128       # j-groups per partition per tile = 64
FREE = JT            # f32 per partition in the interleaved tile

_CACHE = {}


def _build_module(reps=1, variant="computeq", jt=JT, bufs=3, load_eng="sync",
                  store_eng="scalar", copy_split=False, bufs_o=None,
                  copy_ops=4, copy_engs="v", ring_alt=False, faststart=False,
                  shared_pool=False, nst=2, edge_split=4):
    """variant:
      computeq — loads contiguous; DVE copies interleave (per q-half);
                 stores contiguous per q-half on the other HWDGE ring.
                 This is the production configuration.
      load    — interleave happens in the load-DMA dst AP (strided SBUF write)
      compute — loads contiguous; DVE copies interleave; store contiguous
      probe   — no interleave at all (wrong result; empirical DMA roofline)
    """
    import concourse.tile as tile
    from concourse import bacc, mybir

    t_tiles = J // jt
    qt = jt // 128
    free = jt

    nc = bacc.Bacc("TRN2", target_bir_lowering=False, debug=False)
    if variant.startswith("xone"):
        xall = nc.dram_tensor("xall", [t_tiles, NF, 128, qt * D],
                              mybir.dt.float32, kind="ExternalInput").ap()
        xs = None
    else:
        xs = [
            nc.dram_tensor(f"x{i}", [t_tiles, 128, qt, D], mybir.dt.float32,
                           kind="ExternalInput").ap()
            for i in range(NF)
        ]
    out = nc.dram_tensor("out", [t_tiles, 128, free], mybir.dt.float32,
                         kind="ExternalOutput").ap()

    with tile.TileContext(nc) as tc:
        with tc.tile_pool(name="p", bufs=bufs) as pool, \
             tc.tile_pool(name="o", bufs=bufs_o or bufs) as opool:
            ld = getattr(nc, load_eng)
            st = getattr(nc, store_eng)
            for _ in range(reps):
                for t in range(t_tiles):
                    kw_tag = {"tag": "buf"} if shared_pool else {}
                    if ring_alt:
                        ld = st = (nc.sync, nc.scalar)[t % 2]
                    buf = pool.tile([128, free], mybir.dt.float32,
                                    name="buf", **kw_tag)
                    if variant == "load":
                        v = buf[:].rearrange("p (q i d) -> p q i d",
                                             q=qt, i=NF, d=D)
                        for i in range(NF):
                            ld.dma_start(out=v[:, :, i, :], in_=xs[i][t])
                        st.dma_start(out=out[t], in_=buf[:])
                    elif variant == "store":
                        vl = buf[:].rearrange("p (i q d) -> p i q d",
                                              i=NF, q=qt, d=D)
                        for i in range(NF):
                            ld.dma_start(out=vl[:, i], in_=xs[i][t])
                        vs = buf[:].rearrange("p (i q d) -> p q i d",
                                              i=NF, q=qt, d=D)
                        vo = out[t].rearrange("p (q i d) -> p q i d",
                                              q=qt, i=NF, d=D)
                        st.dma_start(out=vo, in_=vs)
                    elif variant == "compute":
                        vl = buf[:].rearrange("p (i q d) -> p i q d",
                                              i=NF, q=qt, d=D)
                        for i in range(NF):
                            ld.dma_start(out=vl[:, i], in_=xs[i][t])
                        obuf = opool.tile([128, free], mybir.dt.float32)
                        vo = obuf[:].rearrange("p (q i d) -> p q i d",
                                               q=qt, i=NF, d=D)
                        vi4 = buf[:].rearrange("p (i q d) -> p q i d",
                                               i=NF, q=qt, d=D)
                        engs = {"v": nc.vector, "s": nc.scalar,
                                "g": nc.gpsimd}
                        step = NF // copy_ops
                        for k in range(copy_ops):
                            eng = engs[copy_engs[k % len(copy_engs)]]
                            lo, hi = k * step, (k + 1) * step
                            if step == 1:
                                eng.tensor_copy(out=vo[:, :, lo, :],
                                                in_=vl[:, lo])
                            else:
                                eng.tensor_copy(
                                    out=vo[:, :, lo:hi, :],
                                    in_=vi4[:, :, lo:hi, :])
                        st.dma_start(out=out[t], in_=obuf[:])
                    elif variant == "computeq":
                        vl = buf[:].rearrange("p (i q d) -> p i q d",
                                              i=NF, q=qt, d=D)
                        if shared_pool:
                            obuf = pool.tile([128, free], mybir.dt.float32,
                                             name="obuf", tag="buf")
                        else:
                            obuf = opool.tile([128, free],
                                              mybir.dt.float32, name="obuf")
                        vo = obuf[:].rearrange("p (q i d) -> p q i d",
                                               q=qt, i=NF, d=D)
                        vi4 = buf[:].rearrange("p (i q d) -> p q i d",
                                               i=NF, q=qt, d=D)
                        # First tile of the program: finer q-granularity so
                        # the first store launches ~3x sooner (one-shot ramp).
                        nsplit = edge_split if (
                            faststart and t in (0, t_tiles - 1)) \
                            else (nst // 2)
                        qh = qt // 2
                        for i in range(NF):
                            for g in range(nsplit):
                                gq = slice(g * qt // nsplit,
                                           (g + 1) * qt // nsplit)
                                ld.dma_start(out=vl[:, i, gq, :],
                                             in_=xs[i][t][:, gq, :])
                        nst_t = 2 * nsplit
                        for h in range(nst_t):
                            qs = slice(h * qt // nst_t,
                                       (h + 1) * qt // nst_t)
                            for k in range(2):
                                nc.vector.tensor_copy(
                                    out=vo[:, qs, 2 * k:2 * k + 2, :],
                                    in_=vi4[:, qs, 2 * k:2 * k + 2, :])
                            st.dma_start(
                                out=out[t][:, h * free // nst_t:
                                           (h + 1) * free // nst_t],
                                in_=obuf[:, h * free // nst_t:
                                         (h + 1) * free // nst_t])
                    elif variant == "hybrid":
                        v = buf[:].rearrange("p (q i d) -> p q i d",
                                             q=qt, i=NF, d=D)
                        for i in range(2):
                            ld.dma_start(out=v[:, :, i, :], in_=xs[i][t])
                        xb = opool.tile([128, free // 2], mybir.dt.float32)
                        vl = xb[:].rearrange("p (i q d) -> p i q d",
                                             i=2, q=qt, d=D)
                        for i in range(2):
                            ld.dma_start(out=vl[:, i], in_=xs[2 + i][t])
                        for i in range(2):
                            eng = nc.vector if (not copy_split or i == 0) \
                                else nc.scalar
                            eng.tensor_copy(out=v[:, :, 2 + i, :],
                                            in_=vl[:, i])
                        st.dma_start(out=out[t], in_=buf[:])
                    elif variant.startswith("xone"):
                        vb = buf[:].rearrange("p (i f) -> p i f",
                                              i=NF, f=qt * D)
                        ld.dma_start(out=vb,
                                     in_=xall[t].rearrange("i p f -> p i f"))
                        obuf = opool.tile([128, free], mybir.dt.float32)
                        vo = obuf[:].rearrange("p (q i d) -> p q i d",
                                               q=qt, i=NF, d=D)
                        vi4 = buf[:].rearrange("p (i q d) -> p q i d",
                                               i=NF, q=qt, d=D)
                        qh = qt // 2
                        n_st = 1 if variant == "xone1s" else 2
                        for h in range(2):
                            qs = slice(h * qh, (h + 1) * qh)
                            for k in range(2):
                                nc.vector.tensor_copy(
                                    out=vo[:, qs, 2 * k:2 * k + 2, :],
                                    in_=vi4[:, qs, 2 * k:2 * k + 2, :])
                            if n_st == 2:
                                st.dma_start(
                                    out=out[t][:, h * free // 2:
                                               (h + 1) * free // 2],
                                    in_=obuf[:, h * free // 2:
                                             (h + 1) * free // 2])
                        if n_st == 1:
                            st.dma_start(out=out[t], in_=obuf[:])
                    elif variant == "probe":
                        vl = buf[:].rearrange("p (i q d) -> p i q d",
                                              i=NF, q=qt, d=D)
                        for i in range(NF):
                            ld.dma_start(out=vl[:, i], in_=xs[i][t])
                        st.dma_start(out=out[t], in_=buf[:])
                    else:
                        raise ValueError(variant)
    nc.compile()
    return nc


def _get_module():
    # faststart: tile 0 runs at q-quarter granularity so the store ring
    # primes ~3x sooner (one-shot ramp); steady-state cost measured ~0.
    if "nc" not in _CACHE:
        _CACHE["nc"] = _build_module(faststart=True)
    return _CACHE["nc"]


def _expected_fold():
    return (np.arange(N) % NF).astype(np.int32)


def kernel(x0, x1, x2, x3, fold):
    xs = [np.asarray(x, dtype=np.float32) for x in (x0, x1, x2, x3)]
    fold = np.asarray(fold)

    if not np.array_equal(fold, _expected_fold()):
        # Fallback: general (host) scatter for a non-standard fold pattern.
        out = np.zeros((fold.shape[0], xs[0].shape[1]), dtype=np.float32)
        for i, x in enumerate(xs):
            idx = np.nonzero(fold == i)[0][: x.shape[0]]
            out[idx] += x
        return out

    from concourse.bass_utils import run_bass_kernel_spmd

    nc = _get_module()
    in_maps = []
    for c in range(NCORES):
        m = {}
        for i, x in enumerate(xs):
            sl = x[c * J:(c + 1) * J]            # [J, 32] contiguous view
            m[f"x{i}"] = np.ascontiguousarray(sl).reshape(T, 128, QT, D)
        in_maps.append(m)

    res = run_bass_kernel_spmd(nc, in_maps, core_ids=list(range(NCORES)))

    out = np.empty((N, D), dtype=np.float32)
    rows = 4 * J                                  # output rows per core
    for c in range(NCORES):
        out[c * rows:(c + 1) * rows] = res.results[c]["out"].reshape(rows, D)
    return out

